# revision 43
# baseline (speedup 1.0000x reference)
"""Difference 3D cost volume on 8 Trainium2 NeuronCores.

cost[n,c,d,h,w] = l[n,c,h,w] - r[n,c,h,w-d]  (w >= d), else 1.0
Shapes: l,r [2,32,128,256] f32 -> out [2,32,48,128,256] f32.

Sharding: data-parallel over the 64 (n,c) slices, 8 per core. Each core
computes, per slice, the [H, D, W] volume in CH-disparity chunks
(broadcast l over d via stride-0 AP, shift -r via stride -1 AP into a
48-col left-padded copy).

Design notes (all measured on HW, see git of _transcript for the hunt):
- Output rounds to bf16 on write-out, halving HBM store traffic; bf16
  keeps rel err <= 2^-9 at every magnitude (fp16 subnormals would blow
  past the 2e-2 gate near the 1e-6 denominator clamp; inputs must stay
  fp32 through the subtract or cancellation destroys small outputs).
- Each chunk is split across two genuinely-parallel pipelines: rows
  [0, CH-PE_K) on DVE (fp32 tensor_add -> bf16), rows [CH-PE_K, CH) on
  TensorE as psum = I @ l + I @ (-r) (accumulating fp32 matmuls, exact
  for +-1 weights; host pre-negates r so both passes share the same
  stationary identity), drained PSUM->SBUF by the Scalar engine with
  the fp32->bf16 round. PE/ACT own SBUF ports; GpSimd shares DVE's
  ports (measured: DVE+GpSimd run at the SUM of their times) so it
  stays off. Splitting inside every chunk keeps PE HAM-warm.
- Chunks store PACKED-TRIMMED: chunk c keeps only w >= d0 = c*CH,
  densely packed in SBUF and DRAM (runs stay multi-KB), cutting both
  store bytes and compute ~7.8%. The host unpacks, upconverts to fp32,
  transposes [h,d] -> [d,h], and fills the 1.0 prefixes (w < d).
- Deep buffering (12 output tiles, all 8 PSUM banks) hides store-DMA
  latency spikes and maximizes PE->ACT drain lookahead; measured ~12%
  over 8-tile/4-bank pools.
Steady-state per-pass ~55 us/core verified (shared-tenancy spread up to
~77 us) vs the 118 us fp32 DVE-only baseline — ~2.2x.
"""

import numpy as np

N, C, H, W, D = 2, 32, 128, 256, 48
PAD = 48  # left pad on r rows; must be >= D
NCORES = 8
PAIRS = N * C
PPC = PAIRS // NCORES  # (n,c) slices per core
CH = 8  # disparities per compute/store chunk (divides D)
OFFLOAD = 0  # if >0, every OFFLOAD-th chunk's subtract runs on GpSimd.
# Measured: GpSimd shares SBUF ports with DVE, so running both gives the
# SUM of their times, not the max — offload only hurts. Keep 0.
PE_K = 2  # disparities per chunk computed on TensorE (the rest on DVE):
# psum = I @ l + I @ (-r) via accumulating fp32 identity matmuls (exact),
# drained to SBUF as bf16 by the Scalar engine. PE/ACT have SBUF ports
# independent of DVE's, so the pipelines genuinely overlap; fp32 matmul
# is 4 cycles/row so k=3 of 8 balances PE against DVE's 5 of 8. Work is
# split WITHIN every chunk so PE never idles long enough to be HAM-
# throttled (~3.4us). float32r (1 cycle/row) would be 4x faster but the
# verifier requires inputs pre-rounded to fp32r's reduced mantissa,
# which breaks near-cancellation outputs — unusable here.
SPLIT_STORES = True  # alternate stores between the SP and ACT HWDGE rings
LEAN_LOADS = False  # load r unpadded (-18% r-load bytes) with a GpSimd
# memset of the 48-column pad; measured no gain (loads are hidden), so
# keep the simpler padded load
TRIM = True  # chunk c only computes/stores columns w >= d0 = c*CH, PACKED
# densely (SBUF tile and DRAM are both [H, ch*(W-d0)] contiguous), so DMA
# runs stay multi-KB while store bytes and compute drop ~7.8%. The host
# unpacks and fills the w < d prefixes with 1.0. (A strided, non-packed
# trim was slower: <512B runs double DMA latency.)


def _chunk_geom(ch=None, trim=None):
    """Per-chunk (d0, wv, packed offset) and total packed width."""
    ch = CH if ch is None else ch
    trim = TRIM if trim is None else trim
    geom = []
    off = 0
    for c in range(D // ch):
        d0 = c * ch if trim else 0
        wv = W - d0
        geom.append((d0, wv, off))
        off += ch * wv
    return geom, off

_nc_cache = None
_runner_cache = None


def _emit(
    tc,
    lf,
    rf,
    out,
    eye=None,
    ch=None,
    offload=None,
    pe_k=None,
    split_stores=None,
    trim=None,
    lean_loads=None,
    loads_on="scalar",
    pe_k_last=None,
    act_every=2,
    op_bufs=16,  # deep output buffering hides store-latency spikes
    # (8->12->16 each measured faster; 16x4KB/partition is still well
    # within SBUF)
    pp_bufs=8,  # all 8 PSUM banks: max PE->ACT drain lookahead
    in_bufs=4,  # l/r tile prefetch depth (slices ahead)
    do_compute=True,
    do_store=True,
    do_load=True,
):
    """Emit the per-core program. lf [PPC,H,W] f32, rf [PPC,H,PAD+W] f32
    holding NEGATED padded r, eye [H,H] f32 identity, out [PPC,H,D,W] bf16
    viewed as [PPC,H,D*W]. do_* flags ablate stages for perf probes
    (output garbage when a do_* flag is off).

    Each chunk's ch disparities are split: the first ch-pe_k rows compute
    on DVE (tensor_add of l and -r, fp32 in, bf16 out), the last pe_k on
    TensorE (psum = I @ l + I @ (-r) via accumulating fp32 matmuls, exact
    for +-1 weights), drained to the same SBUF tile as bf16 by the Scalar
    engine. PE+ACT have SBUF ports independent from DVE's, so the two
    pipelines genuinely overlap; GpSimd shares DVE's ports and is left
    off. Splitting within every chunk keeps PE continuously busy (no
    HAM throttle re-warm).

    With trim, chunk c only computes/stores columns w >= d0 = c*ch (the
    host writes 1.0 over all w < d anyway), saving ~7.8% of compute and
    store bytes — but breaking 4KB store runs; measured slower. Off.
    """
    from concourse import mybir
    from contextlib import ExitStack

    ch = CH if ch is None else ch
    offload = OFFLOAD if offload is None else offload
    pe_k = PE_K if pe_k is None else pe_k
    split_stores = SPLIT_STORES if split_stores is None else split_stores
    trim = TRIM if trim is None else trim
    lean_loads = LEAN_LOADS if lean_loads is None else lean_loads
    # matmul moving operand is capped at 512 fp32 elements; one PSUM bank
    # (512 fp32) per matmul output
    assert 0 <= pe_k < ch
    geom, _ = _chunk_geom(ch, trim)

    nc = tc.nc
    ov = out  # [PPC, H, SPACKED]
    with ExitStack() as ctx:
        lp = ctx.enter_context(tc.tile_pool(name="lp", bufs=in_bufs))
        rp = ctx.enter_context(tc.tile_pool(name="rp", bufs=in_bufs))
        op = ctx.enter_context(tc.tile_pool(name="op", bufs=op_bufs))
        et = None
        if pe_k:
            ep = ctx.enter_context(tc.tile_pool(name="ep", bufs=1))
            pp = ctx.enter_context(
                tc.tile_pool(name="pp", bufs=pp_bufs, space="PSUM")
            )
            et = ep.tile([H, H], mybir.dt.float32)
            nc.sync.dma_start(et[:], eye)
        g = 0  # global chunk counter (store-ring round-robin)
        ld = getattr(nc, loads_on)
        for p in range(PPC):
            lt = lp.tile([H, W], mybir.dt.float32)
            rt = rp.tile([H, PAD + W], mybir.dt.float32)
            if do_load and lean_loads:
                # load r unpadded; GpSimd (otherwise idle) zeroes the pad
                ld.dma_start(lt[:], lf[p])
                ld.dma_start(rt[:, PAD : PAD + W], rf[p][:, PAD : PAD + W])
                nc.gpsimd.memset(rt[:, 0:PAD], 0.0)
            elif do_load:
                ld.dma_start(lt[:], lf[p])
                ld.dma_start(rt[:], rf[p])

            for c in range(D // ch):
                d0, wv, off = geom[c]
                c0 = c * ch  # first disparity of the chunk
                # optionally lighten PE on the slice's last chunk to tune
                # the DVE:PE ratio below pe_k/ch granularity
                k_here = pe_k if (c < D // ch - 1 or pe_k_last is None) \
                    else pe_k_last
                dve_k = ch - k_here  # disparities computed on DVE
                # packed tile: row j (disparity c0+j) at columns [j*wv,(j+1)*wv)
                ot = op.tile([H, ch * wv], mybir.dt.bfloat16, tag="ot")
                g += 1

                if do_compute and dve_k:
                    # ot[h, (j, x)] = l[h, d0+x] + rneg[h, PAD - c0 - j + d0+x]
                    l_ap = lt[:, d0:W]
                    l_ap.ap = l_ap.ap[:-1] + [[0, dve_k], [1, wv]]
                    r_ap = rt[:, PAD - c0 + d0 : PAD - c0 + d0 + wv]
                    r_ap.ap = r_ap.ap[:-1] + [[-1, dve_k], [1, wv]]
                    o_ap = ot[:, 0 : dve_k * wv]
                    o_ap.ap = o_ap.ap[:-1] + [[wv, dve_k], [1, wv]]
                    nc.vector.tensor_add(o_ap, l_ap, r_ap)

                if do_compute and k_here:
                    # psum[h, (j, x)] = l[h, d0+x] + rneg[h, PAD - c0 - j + d0+x]
                    # for j in [dve_k, ch), in per-PSUM-bank groups
                    j0 = dve_k
                    while j0 < ch:
                        md = min(512 // wv, ch - j0)
                        pt = pp.tile([H, md * wv], mybir.dt.float32, tag="pt")
                        l_ap = lt[:, d0:W]
                        l_ap.ap = l_ap.ap[:-1] + [[0, md], [1, wv]]
                        r_ap = rt[:, PAD - c0 - j0 + d0 : PAD - c0 - j0 + d0 + wv]
                        r_ap.ap = r_ap.ap[:-1] + [[-1, md], [1, wv]]
                        nc.tensor.matmul(
                            pt[:], et[:], l_ap, start=True, stop=False
                        )
                        nc.tensor.matmul(
                            pt[:], et[:], r_ap, start=False, stop=True
                        )
                        # ACT drains PSUM -> SBUF, rounding fp32 -> bf16
                        nc.scalar.copy(
                            ot[:, j0 * wv : (j0 + md) * wv], pt[:]
                        )
                        j0 += md

                if do_store:
                    on_act = split_stores and g % act_every == 0
                    st = nc.scalar if on_act else nc.sync
                    st.dma_start(ov[p][:, off : off + ch * wv], ot[:])


def _declare_io(nc, ch=None, trim=None):
    from concourse import mybir

    _, spacked = _chunk_geom(ch, trim)
    lf = nc.dram_tensor("lf", [PPC, H, W], mybir.dt.float32, kind="ExternalInput").ap()
    rf = nc.dram_tensor(
        "rf", [PPC, H, PAD + W], mybir.dt.float32, kind="ExternalInput"
    ).ap()
    eye = nc.dram_tensor(
        "eye", [H, H], mybir.dt.float32, kind="ExternalInput"
    ).ap()
    out = nc.dram_tensor(
        "out", [PPC, H, spacked], mybir.dt.bfloat16, kind="ExternalOutput"
    ).ap()
    return lf, rf, eye, out


def _build():
    global _nc_cache
    if _nc_cache is not None:
        return _nc_cache
    import concourse.tile as tile
    from concourse import bacc

    nc = bacc.Bacc(
        "TRN2", target_bir_lowering=False, debug=False, num_devices=NCORES
    )
    lf, rf, eye, out = _declare_io(nc)
    with tile.TileContext(nc) as tc:
        _emit(tc, lf, rf, out, eye=eye)
    nc.compile()
    _nc_cache = nc
    return nc


def _get_runner():
    """Build (once) a cached PJRT executable over the 8-core mesh.

    No donation: the zero output-operands stay resident on device and are
    reused every call; the NEFF writes every output byte we read back.
    """
    global _runner_cache
    if _runner_cache is not None:
        return _runner_cache

    import jax
    from jax.sharding import Mesh, NamedSharding, PartitionSpec

    import concourse.mybir as mybir
    from concourse.bass2jax import (
        _bass_exec_p,
        install_neuronx_cc_hook,
        partition_id_tensor,
    )

    try:
        from jax.experimental.shard_map import shard_map
    except ImportError:
        from jax.shard_map import shard_map

    nc = _build()
    install_neuronx_cc_hook()
    partition_name = nc.partition_id_tensor.name if nc.partition_id_tensor else None

    in_names, out_names, out_avals, zero_outs = [], [], [], []
    for alloc in nc.m.functions[0].allocations:
        if not isinstance(alloc, mybir.MemoryLocationSet):
            continue
        name = alloc.memorylocations[0].name
        if alloc.kind == "ExternalInput":
            if name != partition_name:
                in_names.append(name)
        elif alloc.kind == "ExternalOutput":
            shape = tuple(alloc.tensor_shape)
            dtype = mybir.dt.np(alloc.dtype)
            out_names.append(name)
            out_avals.append(jax.core.ShapedArray(shape, dtype))
            zero_outs.append(np.zeros(shape, dtype))
    all_in_names = list(in_names) + list(out_names)
    if partition_name is not None:
        all_in_names.append(partition_name)

    def _body(*args):
        operands = list(args)
        if partition_name is not None:
            operands.append(partition_id_tensor())
        outs = _bass_exec_p.bind(
            *operands,
            out_avals=tuple(out_avals),
            in_names=tuple(all_in_names),
            out_names=tuple(out_names),
            lowering_input_output_aliases=(),
            sim_require_finite=False,
            sim_require_nnan=False,
            nc=nc,
        )
        return tuple(outs)

    devices = jax.devices()[:NCORES]
    mesh = Mesh(np.asarray(devices), ("core",))
    nin = len(in_names)
    nout = len(out_names)
    fn = jax.jit(
        shard_map(
            _body,
            mesh=mesh,
            in_specs=(PartitionSpec("core"),) * (nin + nout),
            out_specs=(PartitionSpec("core"),) * nout,
            check_rep=False,
        ),
        keep_unused=True,
    )
    sharding = NamedSharding(mesh, PartitionSpec("core"))
    zeros_dev = [
        jax.device_put(
            np.zeros((NCORES * z.shape[0], *z.shape[1:]), z.dtype), sharding
        )
        for z in zero_outs
    ]
    _runner_cache = (fn, in_names, zeros_dev, sharding)
    return _runner_cache


def _prep_inputs(l_fmap, r_fmap):
    l = np.ascontiguousarray(np.asarray(l_fmap, dtype=np.float32)).reshape(
        PAIRS, H, W
    )
    r = np.ascontiguousarray(np.asarray(r_fmap, dtype=np.float32)).reshape(
        PAIRS, H, W
    )
    # r is shipped NEGATED so both engines ADD it: DVE uses tensor_add and
    # the PE path accumulates two matmuls with the same +identity weights
    # (no weight swap between the l and r passes).
    rpad = np.zeros((PAIRS, H, PAD + W), np.float32)
    rpad[:, :, PAD:] = -r
    eye = np.tile(np.eye(H, dtype=np.float32), (NCORES, 1))
    return {"lf": l, "rf": rpad, "eye": eye}


def in_maps_for(named):
    """Split full input arrays into NCORES per-core dicts (axis-0 shards)."""
    maps = []
    for c in range(NCORES):
        m = {}
        for k, v in named.items():
            n = v.shape[0] // NCORES
            m[k] = np.ascontiguousarray(v[c * n : (c + 1) * n])
        maps.append(m)
    return maps


def _gather(out_global):
    """[PAIRS,H,SPACKED] bf16 device result -> [N,C,D,H,W] f32 with 1.0
    prefixes. Chunk c holds disparities [c*CH,(c+1)*CH) at columns
    w >= d0 = c*CH (packed)."""
    geom, spacked = _chunk_geom()
    full = np.asarray(out_global).astype(np.float32).reshape(N, C, H, spacked)
    out = np.full((N, C, D, H, W), 1.0, np.float32)
    for c, (d0, wv, off) in enumerate(geom):
        seg = full[:, :, :, off : off + CH * wv].reshape(N, C, H, CH, wv)
        out[:, :, c * CH : (c + 1) * CH, :, d0:] = np.moveaxis(seg, 2, 3)
    for d in range(1, D):
        out[:, :, d, :, :d] = 1.0
    return out


def kernel(l_fmap, r_fmap):
    import jax

    fn, in_names, zeros_dev, sharding = _get_runner()
    named = _prep_inputs(l_fmap, r_fmap)
    concat_in = [jax.device_put(named[name], sharding) for name in in_names]
    out_arrs = fn(*concat_in, *zeros_dev)
    return _gather(out_arrs[0])


def run(l_fmap, r_fmap, trace=False):
    """Legacy path via run_bass_kernel_spmd (used for debugging)."""
    from concourse.bass_utils import run_bass_kernel_spmd

    named = _prep_inputs(l_fmap, r_fmap)
    in_maps = in_maps_for(named)
    nc = _build()
    res = run_bass_kernel_spmd(
        nc, in_maps, core_ids=list(range(NCORES)), trace=trace
    )
    parts = [res.results[k]["out"] for k in range(NCORES)]
    out = _gather(np.concatenate(parts, axis=0))
    return out, res


def _emit_kw_io(emit_kw):
    """IO-affecting kwargs (out tensor geometry) from an emit kwargs dict."""
    return {k: emit_kw[k] for k in ("ch", "trim") if k in emit_kw}
